# revision 3
# baseline (speedup 1.0000x reference)
"""LIF spike recurrence kernel for Trainium2 (8 NeuronCores, SPMD). v8.

Problem: x [32, 128, 32, 32, 8] f32, recurrence over last (time) dim:
    u_t = TAU * u_{t-1} * (1 - o_{t-1}) + x_t
    o_t = 1[u_t - VTH > 0]
Output: o [32, 128, 32, 32, 8] f32 (0.0 / 1.0 spikes).

Design (v6 was DVE-bound at 81.6us: DVE busy 64.7us; the input stream rides
one HWDGE ring at the 365 GB/s per-core HBM peak and lands by 49us exec):
  - Shard batch (32) across 8 cores -> 4/core; host pre-transposes each shard
    to plane-major [P=128, T=8, NPP=4096] so every SBUF access is contiguous.
  - v8: columns are split between two independent per-column chains (the
    recurrence is serial per column, so each engine must own whole columns;
    Pool rejects scalar_tensor_tensor at the ISA level but tensor_scalar
    with compares and tensor_tensor add/mult are HW-verified exact):
      DVE  cols [K, NPP):  c = (u<=VTH)*u      (STT is_le/mult)
                           u' = c*TAU + x      (STT mult/add, in place)
      Pool cols [0, K):    m = (u<=VTH)*TAU    (TS  is_le/mult -> {0,TAU})
                           c = m*u             (TT  mult)
                           u' = c + x          (TT  add, in place)
    All steps are exact: masks in {0,TAU}/{0,1}, TAU=2^-2, single rounding
    on the +x, bit-identical to the reference.
    Pool gets the FIRST columns: its chain starts as soon as the head of
    plane 1 lands, while the faster DVE catches up on the tail columns.
  - Spike via ScalarE: o8_t = Sign(u_t - VTH) -> int8, full plane, hidden
    under the compute engines; output DMA rides the Scalar HWDGE ring.
  - Plane 7 arrives in quarters; tail compute+sign per engine sub-chunk
    (each engine signs its own tail chunks: DVE/Pool tensor_scalar is_gt ->
    int8 {0,1}; ScalarE Sign int8 elsewhere; host maps >0 so both work).
  - Host maps >0 to 1.0f (exact). int8 output cuts out-DMA 4x vs f32.
"""

import numpy as np

TAU = 0.25
VTH = 0.3
N_CORES = 8
P = 128
T = 8
B_LOC = 4  # batches per core
PIX_PER_CORE = B_LOC * 128 * 32 * 32  # 524288
NPP = PIX_PER_CORE // P  # 4096 pixels per partition

_CACHE = {}

# Config key, A/B-tested on hardware. Fields:
#   k<j>   : Pool owns columns [0,j), DVE [j,NPP)
#   noaeb  : barrier only {Pool->Activation} instead of all-engine
CFG = "v8_k1136"


def _parse(key):
    k = 1136
    for tok in key.split("_"):
        if tok.startswith("k") and tok[1:].isdigit():
            k = int(tok[1:])
    return dict(k=k, noaeb="noaeb" in key)


def _isect(lo, hi, bounds):
    """Sub-ranges of [lo,hi) cut at the given ascending bounds."""
    cuts = sorted({lo, hi, *[b for b in bounds if lo < b < hi]})
    return list(zip(cuts[:-1], cuts[1:]))


def _build_nc(key=None):
    if key is None:
        key = CFG
    cfg = _parse(key)
    import concourse.tile as tile
    from concourse import bacc, mybir

    f32 = mybir.dt.float32
    i8 = mybir.dt.int8
    Alu = mybir.AluOpType
    AF = mybir.ActivationFunctionType

    nc = bacc.Bacc(
        "TRN2",
        target_bir_lowering=False,
        debug=False,
        enable_asserts=False,
        num_devices=N_CORES,
    )
    x_d = nc.dram_tensor("x", [P, T, NPP], f32, kind="ExternalInput").ap()
    o_d = nc.dram_tensor("o", [P, T, NPP], i8, kind="ExternalOutput").ap()

    # ACT activation bias needs a pre-registered const AP.
    cb = nc.alloc_sbuf_tensor("const-f32-negvth", [128, 1], f32)
    nc.gpsimd.memset(cb.ap(), -VTH)
    nc.const_aps.aps[(f32, -VTH)] = cb.ap()
    if cfg["noaeb"]:
        # Only ScalarE reads cb, and Pool's own compute is in-order after the
        # memset; barrier just {GpSimd -> Scalar} so Sync enqueues DMA sooner.
        nc.multi_engine_barrier(
            [mybir.EngineType.Pool, mybir.EngineType.Activation]
        )
    else:
        nc.all_engine_barrier()

    K = cfg["k"]                          # Pool/DVE column boundary
    head_bounds = [0, K, 2048, NPP]       # planes 0/1 DMA + step-1 chunks
    q7 = [0, 1024, 2048, 3072, NPP]       # plane-7 DMA quarters

    with tile.TileContext(nc) as tc:
        with tc.tile_pool(name="pp", bufs=1) as pp:
            xt = pp.tile([P, T, NPP], f32, tag="xt")
            c = pp.tile([P, NPP], f32, tag="c")
            m = pp.tile([P, K], f32, tag="m")
            o8 = pp.tile([P, T, NPP], i8, tag="o8")

            # --- input DMA enqueues, all on the Sync HWDGE ring ---
            # Planes 0/1 interleaved in column-chunks (earlier step-1 start);
            # one cut at K so each engine's range completes separately.
            for lo, hi in _isect(0, NPP, head_bounds):
                nc.sync.dma_start(xt[:, 0, lo:hi], x_d[:, 0, lo:hi])
                nc.sync.dma_start(xt[:, 1, lo:hi], x_d[:, 1, lo:hi])
            for t in range(2, T - 1):
                nc.sync.dma_start(xt[:, t, :], x_d[:, t, :])
            # Plane 7 in quarters so tail compute starts per-chunk.
            for lo, hi in _isect(0, NPP, [*q7, K]):
                nc.sync.dma_start(xt[:, 7, lo:hi], x_d[:, 7, lo:hi])

            def cu_dve(t, lo, hi):
                sl = slice(lo, hi)
                up = xt[:, t - 1, sl]
                nc.vector.scalar_tensor_tensor(
                    c[:, sl], up, VTH, up, op0=Alu.is_le, op1=Alu.mult
                )
                nc.vector.scalar_tensor_tensor(
                    xt[:, t, sl], c[:, sl], TAU, xt[:, t, sl],
                    op0=Alu.mult, op1=Alu.add,
                )

            def cu_pool(t, lo, hi):
                sl = slice(lo, hi)
                up = xt[:, t - 1, sl]
                nc.gpsimd.tensor_scalar(
                    m[:, sl], up, VTH, TAU, op0=Alu.is_le, op1=Alu.mult
                )
                nc.gpsimd.tensor_tensor(c[:, sl], m[:, sl], up, op=Alu.mult)
                nc.gpsimd.tensor_tensor(
                    xt[:, t, sl], c[:, sl], xt[:, t, sl], op=Alu.add
                )

            # Plane 0: u_0 = x_0, spike immediately.
            nc.scalar.activation(o8[:, 0, :], xt[:, 0, :], AF.Sign, bias=-VTH)
            nc.scalar.dma_start(o_d[:, 0, :], o8[:, 0, :])

            # Step 1: chunked per engine (chases the chunked DMAs).
            for lo, hi in _isect(0, K, head_bounds):
                cu_pool(1, lo, hi)
            for lo, hi in _isect(K, NPP, head_bounds):
                cu_dve(1, lo, hi)
            nc.scalar.activation(o8[:, 1, :], xt[:, 1, :], AF.Sign, bias=-VTH)
            nc.scalar.dma_start(o_d[:, 1, :], o8[:, 1, :])

            # Steps 2..T-2: one chunk per engine, full-plane sign on ScalarE.
            for t in range(2, T - 1):
                cu_pool(t, 0, K)
                cu_dve(t, K, NPP)
                nc.scalar.activation(o8[:, t, :], xt[:, t, :], AF.Sign, bias=-VTH)
                nc.scalar.dma_start(o_d[:, t, :], o8[:, t, :])

            # Step T-1: per-chunk compute+sign+out, each engine signs its own
            # chunks with tensor_scalar is_gt (int8 {0,1}).
            def tail(eng, cu, lo, hi):
                cu(T - 1, lo, hi)
                sl = slice(lo, hi)
                eng.tensor_scalar(
                    o8[:, T - 1, sl], xt[:, T - 1, sl], VTH, None, op0=Alu.is_gt
                )
                nc.scalar.dma_start(o_d[:, T - 1, sl], o8[:, T - 1, sl])

            for lo, hi in _isect(0, K, q7):
                tail(nc.gpsimd, cu_pool, lo, hi)
            for lo, hi in _isect(K, NPP, q7):
                tail(nc.vector, cu_dve, lo, hi)
    nc.compile()
    return nc


def _get_nc(key=None):
    k = key or CFG
    if k not in _CACHE:
        _CACHE[k] = _build_nc(k)
    return _CACHE[k]


def _shard(x: np.ndarray):
    xs = np.ascontiguousarray(x, dtype=np.float32)
    return [
        np.ascontiguousarray(
            xs[i * B_LOC : (i + 1) * B_LOC].reshape(P, NPP, T).transpose(0, 2, 1)
        )
        for i in range(N_CORES)
    ]


def _run(in_maps, key=None, **kwargs):
    from concourse.bass_utils import run_bass_kernel_spmd

    nc = _get_nc(key)
    return run_bass_kernel_spmd(nc, in_maps, core_ids=list(range(N_CORES)), **kwargs)


def kernel(x: np.ndarray) -> np.ndarray:
    in_maps = [{"x": s} for s in _shard(x)]
    res = _run(in_maps)
    outs = []
    for i in range(N_CORES):
        s8 = res.results[i]["o"]  # [P, T, NPP] int8 sign / {0,1} values
        o = (s8 > 0).transpose(0, 2, 1).astype(np.float32)  # [P, NPP, T]
        outs.append(o.reshape(B_LOC, 128, 32, 32, T))
    return np.concatenate(outs, axis=0)


# revision 7
# speedup vs baseline: 2.1322x; 2.1322x over previous
"""LIF spike recurrence kernel for Trainium2 (8 NeuronCores, SPMD). v10.

Problem: x [32, 128, 32, 32, 8] f32, recurrence over last (time) dim:
    u_t = TAU * u_{t-1} * (1 - o_{t-1}) + x_t
    o_t = 1[u_t - VTH > 0]
Output: o [32, 128, 32, 32, 8] f32 (0.0 / 1.0 spikes).

Design (v6 was DVE-bound at 81.6us: 14 full-plane STT ops = 62us serial on
DVE while the input stream lands by 49us exec; Pool/GpSimd ALU is slow
Q7 software that also poisons DVE SBUF bandwidth, so the only usable second
ALU is the PE + ScalarE combination):
  - Shard batch (32) across 8 cores -> 4/core; host pre-transposes each shard
    to plane-major [P=128, T=8, NPP=4096] and PRE-SCALES plane t by 4^t
    (exact power-of-two), switching to w-space: w_t = u_t * 4^t obeys
        w_t = w_{t-1} * [w_{t-1} <= VTH*4^(t-1)] + x'_t,  x'_t = x_t * 4^t
    which removes the TAU multiply (power-of-two scaling commutes with fp32
    rounding, so this is bit-exact vs the reference).
  - Columns [0, W): PSUM chain (W=2048 fills the 16KB/partition PSUM with
    two ping-pong regions). Per step:
        DVE : c = (w_prev <= TH) * w_prev       (STT, SBUF->SBUF)
        PE  : psum  = I.T @ x'_t    (start)     \ 4 matmuls of 512 cols
        PE  : psum += I.T @ c       (stop)      / = x'_t + c; HW-probe-
                                                  verified BIT-EXACT fp32
        SE  : xt[:,t,0:W] = Copy(psum)          (ACT, scale=1 -> exact;
                                                 lands w_t where x'_t was,
                                                 which the PE already read)
  - Columns [W, NPP): classic SBUF chain on DVE:
        c = (w_prev <= TH) * w_prev             (STT)
        w = c + x'_t  in place                  (TT add)
    DVE per step does W + 2*(NPP-W) columns instead of 2*NPP: 62us -> 48us,
    with PE absorbing the adds and SE the copy-back for half the plane.
  - Spike via ScalarE: one full-plane o8 = Sign(w * 4^-t - VTH) -> int8
    (scale is an exact power of two, so this is the reference's u_t - VTH
    compare); output DMA rides the Scalar HWDGE ring. Sign and Copy live in
    the same ACT function set (no table reloads).
  - Host maps >0 to 1.0f (exact). int8 output cuts out-DMA 4x vs f32.
"""

import numpy as np

TAU = 0.25
VTH = 0.3
N_CORES = 8
P = 128
T = 8
B_LOC = 4  # batches per core
PIX_PER_CORE = B_LOC * 128 * 32 * 32  # 524288
NPP = PIX_PER_CORE // P  # 4096 pixels per partition

_CACHE = {}

# Config key, A/B-tested on hardware. Fields:
#   w<j>   : PSUM-chain columns [0,j) (multiple of 512, <= 2048)
#   noaeb  : barrier only {Pool->Activation} instead of all-engine
CFG = "v10_w2048"


def _parse(key):
    w = 2048
    for tok in key.split("_"):
        if tok.startswith("w") and tok[1:].isdigit():
            w = int(tok[1:])
    return dict(w=w, noaeb="noaeb" in key)


def _isect(lo, hi, bounds):
    """Sub-ranges of [lo,hi) cut at the given ascending bounds."""
    cuts = sorted({lo, hi, *[b for b in bounds if lo < b < hi]})
    return list(zip(cuts[:-1], cuts[1:]))


def _th(t):
    """Spike threshold in w-space at step t (exact power-of-two scaling)."""
    return VTH * (4.0 ** t)


def _build_nc(key=None):
    if key is None:
        key = CFG
    cfg = _parse(key)
    import concourse.tile as tile
    from concourse import bacc, mybir
    from concourse.bass import MemorySpace

    f32 = mybir.dt.float32
    i32 = mybir.dt.int32
    i8 = mybir.dt.int8
    Alu = mybir.AluOpType
    AF = mybir.ActivationFunctionType

    nc = bacc.Bacc(
        "TRN2",
        target_bir_lowering=False,
        debug=False,
        enable_asserts=False,
        num_devices=N_CORES,
    )
    x_d = nc.dram_tensor("x", [P, T, NPP], f32, kind="ExternalInput").ap()
    o_d = nc.dram_tensor("o", [P, T, NPP], i8, kind="ExternalOutput").ap()

    # ACT activation bias needs a pre-registered const AP.
    cb = nc.alloc_sbuf_tensor("const-f32-negvth", [128, 1], f32)
    nc.gpsimd.memset(cb.ap(), -VTH)
    nc.const_aps.aps[(f32, -VTH)] = cb.ap()
    if cfg["noaeb"]:
        nc.multi_engine_barrier(
            [mybir.EngineType.Pool, mybir.EngineType.Activation]
        )
    else:
        nc.all_engine_barrier()

    W = cfg["w"]                               # PSUM-chain column count
    MM = 512                                   # moving cap per matmul
    head_bounds = [0, 1024, 2048, 3072, NPP]   # planes 0/1 DMA chunks
    q7 = [0, 2048, 3072, NPP]                  # plane-7 DMA chunks

    with tile.TileContext(nc) as tc:
        with tc.tile_pool(name="pp", bufs=1) as pp, \
             tc.tile_pool(name="psp", bufs=1, space=MemorySpace.PSUM) as psp:
            xt = pp.tile([P, T, NPP], f32, tag="xt")
            c = pp.tile([P, NPP], f32, tag="c")
            o8 = pp.tile([P, T, NPP], i8, tag="o8")
            cols = pp.tile([P, P], i32, tag="cols")
            pidx = pp.tile([P, 1], i32, tag="pidx")
            colsf = pp.tile([P, P], f32, tag="colsf")
            pidxf = pp.tile([P, 1], f32, tag="pidxf")
            I = pp.tile([P, P], f32, tag="I")
            pu = [
                psp.tile([P, W], f32, name=f"pu{i}", tag=f"pu{i}")
                for i in range(2)
            ]

            # Identity weights for the PE add-passes (probe-verified exact).
            nc.gpsimd.iota(cols, pattern=[[1, P]], channel_multiplier=0)
            nc.gpsimd.iota(pidx, pattern=[[0, 1]], channel_multiplier=1)
            nc.vector.tensor_copy(colsf, cols)
            nc.vector.tensor_copy(pidxf, pidx)
            nc.vector.tensor_scalar(I, colsf, pidxf, None, op0=Alu.is_equal)

            # --- input DMA enqueues, all on the Sync HWDGE ring ---
            # Planes 0/1 interleaved per 1K-chunk so step 1 starts early.
            for lo, hi in _isect(0, NPP, head_bounds):
                nc.sync.dma_start(xt[:, 0, lo:hi], x_d[:, 0, lo:hi])
                nc.sync.dma_start(xt[:, 1, lo:hi], x_d[:, 1, lo:hi])
            for t in range(2, T - 1):
                nc.sync.dma_start(xt[:, t, :], x_d[:, t, :])
            for lo, hi in _isect(0, NPP, q7):
                nc.sync.dma_start(xt[:, 7, lo:hi], x_d[:, 7, lo:hi])

            def stt(t, lo, hi):
                # c[:, lo:hi] = (w_{t-1} <= TH_{t-1}) * w_{t-1}   (SBUF)
                sl = slice(lo, hi)
                up = xt[:, t - 1, sl]
                nc.vector.scalar_tensor_tensor(
                    c[:, sl], up, _th(t - 1), up, op0=Alu.is_le, op1=Alu.mult
                )

            def mm_load(t, lo, hi):
                for s in range(lo, hi, MM):
                    nc.tensor.matmul(
                        pu[t % 2][:, s : s + MM], I, xt[:, t, s : s + MM],
                        start=True, stop=False,
                    )

            def mm_acc(t, lo, hi):
                for s in range(lo, hi, MM):
                    nc.tensor.matmul(
                        pu[t % 2][:, s : s + MM], I, c[:, s : s + MM],
                        start=False, stop=True,
                    )

            def se_copy(t, lo, hi):
                # w_t (psum) -> xt[:, t, lo:hi]; scale=1 bias=0 is exact.
                nc.scalar.activation(
                    xt[:, t, lo:hi], pu[t % 2][:, lo:hi], AF.Copy,
                    bias=0.0, scale=1.0,
                )

            def tt_add(t, lo, hi):
                sl = slice(lo, hi)
                nc.vector.tensor_tensor(
                    xt[:, t, sl], c[:, sl], xt[:, t, sl], op=Alu.add
                )

            def sign(t, lo, hi, out_dma=True):
                sc = 0.25 ** t
                nc.scalar.activation(
                    o8[:, t, lo:hi], xt[:, t, lo:hi], AF.Sign,
                    bias=-VTH, scale=sc,
                )
                if out_dma:
                    nc.scalar.dma_start(o_d[:, t, lo:hi], o8[:, t, lo:hi])

            # Plane 0: w_0 = x'_0 already in SBUF.
            sign(0, 0, NPP)

            # Step 1: chunked to chase the interleaved plane-0/1 DMAs.
            for lo, hi in _isect(0, W, head_bounds):
                stt(1, lo, hi)
                mm_load(1, lo, hi)
                mm_acc(1, lo, hi)
            se_copy(1, 0, W)
            for lo, hi in _isect(W, NPP, head_bounds):
                stt(1, lo, hi)
                tt_add(1, lo, hi)
            sign(1, 0, NPP)

            # Steps 2..6: plane-granular.
            for t in range(2, T - 1):
                stt(t, 0, W)
                mm_load(t, 0, W)
                mm_acc(t, 0, W)
                se_copy(t, 0, W)
                stt(t, W, NPP)
                tt_add(t, W, NPP)
                sign(t, 0, NPP)

            # Step 7: chunked tail; psum cols flow per 1K chunk, sbuf cols
            # chase the chunked plane-7 DMA, signs/outs fire per chunk.
            t = T - 1
            for lo, hi in _isect(0, W, [1024]):
                stt(t, lo, hi)
                mm_load(t, lo, hi)
                mm_acc(t, lo, hi)
                se_copy(t, lo, hi)
                sign(t, lo, hi)
            for lo, hi in _isect(W, NPP, q7):
                stt(t, lo, hi)
                tt_add(t, lo, hi)
                sign(t, lo, hi)
    nc.compile()
    return nc


def _get_nc(key=None):
    k = key or CFG
    if k not in _CACHE:
        _CACHE[k] = _build_nc(k)
    return _CACHE[k]


_WSCALE = (4.0 ** np.arange(T)).astype(np.float32)  # exact powers of two


def _shard(x: np.ndarray):
    xs = np.ascontiguousarray(x, dtype=np.float32) * _WSCALE  # w-space
    return [
        np.ascontiguousarray(
            xs[i * B_LOC : (i + 1) * B_LOC].reshape(P, NPP, T).transpose(0, 2, 1)
        )
        for i in range(N_CORES)
    ]


def _run(in_maps, key=None, **kwargs):
    from concourse.bass_utils import run_bass_kernel_spmd

    nc = _get_nc(key)
    return run_bass_kernel_spmd(nc, in_maps, core_ids=list(range(N_CORES)), **kwargs)


def kernel(x: np.ndarray) -> np.ndarray:
    in_maps = [{"x": s} for s in _shard(x)]
    res = _run(in_maps)
    outs = []
    for i in range(N_CORES):
        s8 = res.results[i]["o"]  # [P, T, NPP] int8 sign values
        o = (s8 > 0).transpose(0, 2, 1).astype(np.float32)  # [P, NPP, T]
        outs.append(o.reshape(B_LOC, 128, 32, 32, T))
    return np.concatenate(outs, axis=0)
